# revision 5
# baseline (speedup 1.0000x reference)
"""Trainium2 Bass kernel for nn_DiffPoolPrompt (GCN conv + softmax pooling prompt).

Reference computation:
    h = x + sum(cluster_emb, 0)
    logits = GCNConv(h, W, bias, edge_index)   # sym-normalized, self-loops
    s = softmax(logits, axis=1)
    out = x + s @ cluster_emb

Distribution (8 NeuronCores), final:
  - Nodes sharded contiguously: core c owns nodes [c*12500, (c+1)*12500);
    x ships fp16 in natural order (no host permutes of x or out).
  - x is transposed once on-device via the XBAR DMA transpose; each core
    computes g = dinv * (x@W + cW) in fp16 (3 matmuls/row-block, f32 PSUM),
    writes a packed [12544, 10] fp16 bounce, AllGathers it (2MB total), then
    expands into a 256B-strided gather table partitioned into 4 windows
    (int16 idx limit).  Window w holds natural ranks [w*3136, (w+1)*3136) of
    every core plus one forced-zero row for padding slots.
  - Edges partitioned by destination core.  Destination slots use a per-core
    degree-sort permutation (sigma) so rank-rows are degree-homogeneous,
    minimizing rectangular-bucket padding.  Uniform-slot-count gather chunks
    (SLACK-bounded greedy row groups) let ONE dma_gather + ONE 4D strided
    reduce per chunk do the segment-sum on-chip.  NOTE: each window's idx
    stream must be DMA'd into its SBUF tile immediately before that window's
    gathers (program order defines the RAW dependency when win tiles rotate).
  - softmax(logits) runs in sigma layout; s is un-permuted back to natural
    order with one small on-device gather through a local strided table.
  - p = s @ emb per row-block; out = x + p written fp16, 7 row-blocks per
    DMA; host converts to f32.  dinv is shipped in BOTH natural order (for
    g) and sigma order (for logits).
"""

import hashlib
import numpy as np

import concourse.bass as bass
import concourse.bacc as bacc
import concourse.tile as tile
import concourse.mybir as mybir
from concourse.bass_utils import run_bass_kernel_spmd
from concourse.masks import make_identity
from concourse import ap_utils

N_NODES = 100000
N_EDGES = 3200000
IN_CH = 256
K = 10
NCORES = 8
P = 128
NPC = N_NODES // NCORES          # 12500 nodes per core
R = (NPC + P - 1) // P           # 98 rank-rows per core
NPAD = P * R                     # 12544 padded ranks per core
NWIN = 4                         # gather windows (int16 idx limit)
QW = NPAD // NWIN                # 3136 ranks of each core per window
WINROWS = NCORES * QW            # 25088 real table rows per window
TROWS = WINROWS + 1              # + forced-zero row at index 25088
ZPAD = WINROWS                   # pad slots point here
GCOLS = 120                      # gather chunk: 120 cols * 128 = 15360 tokens

F32 = mybir.dt.float32
F16 = mybir.dt.float16
I16 = mybir.dt.int16


def _raw_dma_gather(gp, out_ap, in_ap, idxs_ap, num_idxs, elem_size, elem_step,
                    queue_num=0):
    """bass.dma_gather minus the 256B-elem assert (non-transpose, DRAM src)."""
    assert idxs_ap.dtype == mybir.dt.int16
    assert in_ap.space == bass.MemorySpace.DRAM
    assert idxs_ap.space == bass.MemorySpace.SBUF
    assert out_ap.space == bass.MemorySpace.SBUF
    assert in_ap.dtype == out_ap.dtype
    assert ap_utils.ap_is_contiguous(in_ap.ap[1:])
    assert ap_utils.ap_is_contiguous(out_ap.ap[1:])
    assert ap_utils.ap_is_contiguous(idxs_ap.ap[1:])
    assert in_ap.ap[-1][1] == elem_size and out_ap.ap[-1][1] == elem_size
    assert in_ap.ap[0][0] == elem_step
    stride_bytes = elem_step * mybir.dt.size(in_ap.dtype)
    stride_bytes_256 = stride_bytes // 256
    assert stride_bytes_256 * 256 == stride_bytes and 0 < stride_bytes_256 < 256
    _in_ap = gp.lower_ap_dma(in_ap, for_custom_bir_dma=True)
    _idxs_ap = gp.lower_ap(idxs_ap)
    _out_ap = gp.lower_ap(out_ap)
    return gp.add_instruction(
        mybir.InstDMAGatherAnt(
            name=gp.bass.get_next_instruction_name(),
            ins=[*_in_ap, _idxs_ap, gp.lower_val_access(gp.to_reg(num_idxs))],
            outs=[_out_ap],
            transpose=False,
            num_idxs=num_idxs,
            elem_size=elem_size,
            stride_bytes_256=stride_bytes_256,
            gen_mode=0,
            single_packet=False,
            queue_num=queue_num,
            sbuf_tokens_per_rank=0,
            sbuf_free_dim_per_rank=0,
            sbuf_free_dim_pad_per_rank=0,
            sbuf_byte_offset=0,
        )
    )


# ----------------------------------------------------------------------------
# Host-side sharding / index prep (numpy, index-only)
# ----------------------------------------------------------------------------

def plan_chunks(K_w):
    """Greedy row grouping per window with uniform slot count kk per chunk.
    Returns (chunks, row_coloff, total_cols):
      chunks: list of (w, r0, nrows, kk, col0)
      row_coloff[w, r]: global column offset of row r's slots (-1 if empty)
    """
    K_w = np.asarray(K_w, dtype=np.int64)
    chunks = []
    row_coloff = np.full((NWIN, R), -1, dtype=np.int64)
    col = 0
    for w in range(NWIN):
        r = 0
        while r < R:
            if int(K_w[w, r]) == 0:
                r += 1
                continue
            SLACK = 8
            kk = 0
            nrows = 0
            r2 = r
            while r2 < R:
                kk2 = max(kk, int(K_w[w, r2]))
                extra = (r2 - r + 1) * kk2 - (r2 - r) * kk - int(K_w[w, r2])
                if (r2 - r + 1) * kk2 > GCOLS or (nrows > 0 and extra > SLACK):
                    break
                kk = kk2
                nrows = r2 - r + 1
                r2 += 1
            chunks.append((w, r, nrows, kk, col))
            for i in range(nrows):
                row_coloff[w, r + i] = col + i * kk
            col += nrows * kk
            r += nrows
    return chunks, row_coloff, col


def host_prep(edge_index):
    """Partition + sort edges, build per-core windowed gather index streams."""
    src = np.asarray(edge_index[0], dtype=np.int64)
    dst = np.asarray(edge_index[1], dtype=np.int64)

    deg_all = np.bincount(dst, minlength=N_NODES).astype(np.float32) + 1.0

    # append self-loops
    loop = np.arange(N_NODES, dtype=np.int64)
    src = np.concatenate([src, loop])
    dst = np.concatenate([dst, loop])

    # per-core degree-sort permutation: sigma rank of node v (dst side only)
    rank_sigma = np.empty(N_NODES, dtype=np.int64)
    sig_orders = []
    for cc in range(NCORES):
        ds = deg_all[cc * NPC:(cc + 1) * NPC]
        order = np.argsort(-ds, kind="stable")
        sig_orders.append(order)
        rank_sigma[cc * NPC + order] = np.arange(NPC)

    # dst decomposition: core, partition, row (in sigma space)
    c = dst // NPC
    sr = rank_sigma[dst]
    p = sr % P
    r = sr // P
    # src decomposition: window + within-window table row
    cs = src // NPC
    us = src - cs * NPC
    w = us // QW
    twr = (cs * QW + us - w * QW).astype(np.int16)

    key = ((c * NWIN + w) * R + r) * P + p
    order = np.argsort(key, kind="stable")
    key_s = key[order]
    twr_s = twr[order]

    cnt = np.bincount(key_s, minlength=NCORES * NWIN * R * P)
    cnt4 = cnt.reshape(NCORES, NWIN, R, P)
    K_w = cnt4.max(axis=(0, 3)).astype(np.int64)          # [NWIN, R]
    assert K_w.max() <= GCOLS, f"bucket overflow: {K_w.max()} > {GCOLS}"

    chunks, row_coloff, total_cols = plan_chunks(K_w)
    TOT = total_cols * P
    TOT16 = TOT // 16
    # window w's column span (chunks of a window are contiguous)
    wcol0 = np.zeros(NWIN + 1, dtype=np.int64)
    for (w, r0, nrows, kk, col0) in chunks:
        wcol0[w + 1] = max(wcol0[w + 1], col0 + nrows * kk)
    for w in range(NWIN):
        wcol0[w + 1] = max(wcol0[w + 1], wcol0[w])

    # within-bucket slot number for every (sorted) edge
    starts = np.concatenate([[0], np.cumsum(cnt)])[:-1]
    k_within = np.arange(key_s.shape[0], dtype=np.int64) - starts[key_s]

    # token position inside the full stream (per core)
    wk = (key_s // (R * P)) % NWIN
    rk = (key_s // P) % R
    pk = key_s % P
    t = (row_coloff[wk, rk] + k_within) * P + pk

    core_of = key_s // (NWIN * R * P)
    core_starts = np.searchsorted(core_of, np.arange(NCORES + 1))

    idxs = []
    degs = []
    degs_nat = []
    uidxs = []
    for cc in range(NCORES):
        lo, hi = core_starts[cc], core_starts[cc + 1]
        flat = np.full(TOT, ZPAD, dtype=np.int16)
        flat[t[lo:hi]] = twr_s[lo:hi]
        wrap = flat.reshape(TOT16, 16).T                   # [16, TOT16]
        idxs.append(np.ascontiguousarray(wrap))

        dg = np.ones(NPAD, dtype=np.float32)
        dg[:NPC] = deg_all[cc * NPC:(cc + 1) * NPC][sig_orders[cc]]
        degs.append(np.ascontiguousarray(dg.reshape(R, P).T))  # [P, R] sigma
        dn = np.ones(NPAD, dtype=np.float32)
        dn[:NPC] = deg_all[cc * NPC:(cc + 1) * NPC]
        degs_nat.append(np.ascontiguousarray(dn.reshape(R, P).T))

        # un-permute gather stream: natural rank u reads sigma-rank row of u
        flat_u = np.arange(NPAD, dtype=np.int16)
        flat_u[:NPC] = rank_sigma[cc * NPC:(cc + 1) * NPC]
        wrap_u = flat_u.reshape(NPAD // 16, 16).T
        uidxs.append(np.ascontiguousarray(
            np.concatenate([wrap_u, wrap_u], axis=0)))     # [32, 784]

    return {"deg": degs, "deg_nat": degs_nat, "idx": idxs, "uidx": uidxs,
            "K_w": K_w, "TOT16": TOT16}


_PREP_CACHE = {}


def host_prep_cached(edge_index):
    e = np.asarray(edge_index)
    samp = e[:, ::997].tobytes()
    fp = (e.shape, str(e.dtype), int(e.sum()),
          hashlib.blake2b(samp, digest_size=16).hexdigest())
    hit = _PREP_CACHE.get(fp)
    if hit is None:
        hit = host_prep(e)
        _PREP_CACHE.clear()
        _PREP_CACHE[fp] = hit
    return hit


# ----------------------------------------------------------------------------
# Device kernel
# ----------------------------------------------------------------------------

_BUILD_CACHE = {}

# experiment knobs (overridden by prof scripts; defaults = production)
NQUEUES = 1
PHASES = frozenset({"B", "AG", "GATHER", "SOFTMAX", "D"})


def build_kernel(K_w, TOT16):
    K_w = np.asarray(K_w, dtype=np.int64)
    key = (TOT16, NQUEUES, PHASES) + tuple(int(k) for k in K_w.ravel())
    if key in _BUILD_CACHE:
        return _BUILD_CACHE[key]
    chunks, row_coloff, total_cols = plan_chunks(K_w)
    assert total_cols * P == TOT16 * 16
    wtok0 = [None] * NWIN
    wtokend = [0] * NWIN
    for (w, r0, nrows, kk, col0) in chunks:
        if wtok0[w] is None:
            wtok0[w] = col0 * P
        wtokend[w] = (col0 + nrows * kk) * P
    T16s = [(wtokend[w] - (wtok0[w] or 0)) // 16 for w in range(NWIN)]

    nc = bacc.Bacc("TRN2", target_bir_lowering=False, debug=False,
                   num_devices=NCORES, num_swdge_queues=NQUEUES)

    x_in = nc.dram_tensor("x", [NPAD, IN_CH], F16, kind="ExternalInput").ap()
    w_in = nc.dram_tensor("w", [IN_CH, K], F16, kind="ExternalInput").ap()
    bias_in = nc.dram_tensor("bias", [1, K], F32, kind="ExternalInput").ap()
    emb_in = nc.dram_tensor("emb", [K, IN_CH], F16, kind="ExternalInput").ap()
    deg_in = nc.dram_tensor("deg", [P, R], F32, kind="ExternalInput").ap()
    degn_in = nc.dram_tensor("degn", [P, R], F32, kind="ExternalInput").ap()
    idx_in = nc.dram_tensor("idx", [16, TOT16], I16, kind="ExternalInput").ap()
    uidx_in = nc.dram_tensor("uidx", [32, NPAD // 16], I16,
                             kind="ExternalInput").ap()
    out = nc.dram_tensor("out", [NPAD, IN_CH], F16, kind="ExternalOutput").ap()

    with tile.TileContext(nc) as tc:
        with tc.tile_pool(name="big", bufs=1) as big, \
             tc.tile_pool(name="small", bufs=1) as small, \
             tc.tile_pool(name="xt", bufs=4) as xtp, \
             tc.tile_pool(name="msg", bufs=4) as msgp, \
             tc.tile_pool(name="ops", bufs=3) as opsp, \
             tc.tile_pool(name="ops2", bufs=3) as opsp2, \
             tc.tile_pool(name="ps0", bufs=1, space="PSUM") as ps0, \
             tc.tile_pool(name="psHW", bufs=3, space="PSUM") as psHW, \
             tc.tile_pool(name="psST", bufs=2, space="PSUM") as psST, \
             tc.tile_pool(name="psP", bufs=2, space="PSUM") as psP, \
             tc.tile_pool(name="dram", bufs=1, space="DRAM") as dram:

            # ---- resident loads: x transposed once via the XBAR
            xT = big.tile([P, 2 * NPAD], F16)             # [ch%128, ch//128, n]
            for q in range(4):
                nc.sync.dma_start_transpose(
                    xT[:].rearrange("p (h n) -> p h n", n=NPAD)
                        [:, :, q * QW:(q + 1) * QW],
                    x_in[q * QW:(q + 1) * QW, :])
            w_sb = small.tile([P, 2 * K], F16)            # [ch%128, 2 chunks]
            nc.sync.dma_start(w_sb[:, 0:K], w_in[0:P, :])
            nc.sync.dma_start(w_sb[:, K:2 * K], w_in[P:2 * P, :])
            emb_sb = small.tile([K, IN_CH], F16)
            nc.sync.dma_start(emb_sb[:], emb_in[:])
            deg_sb = small.tile([P, R], F32)
            nc.sync.dma_start(deg_sb[:], deg_in[:])
            degn_sb = small.tile([P, R], F32)
            nc.sync.dma_start(degn_sb[:], degn_in[:])
            bias_sb = small.tile([1, K], F32)
            nc.sync.dma_start(bias_sb[:], bias_in[:])

            ident_h = small.tile([P, P], F16)
            make_identity(nc, ident_h[:])

            ones_row_h = small.tile([1, P], F16)
            nc.vector.memset(ones_row_h[:], 1.0)
            ones_row_f = small.tile([1, P], F32)
            nc.vector.memset(ones_row_f[:], 1.0)
            ones_col10 = small.tile([K, 1], F16)
            nc.vector.memset(ones_col10[:], 1.0)

            # ---- dinv = 1/sqrt(deg): sigma order (logits) + natural (g)
            dinv_sb = small.tile([P, R], F32)
            nc.scalar.activation(dinv_sb[:], deg_sb[:],
                                 mybir.ActivationFunctionType.Sqrt)
            nc.vector.reciprocal(dinv_sb[:], dinv_sb[:])
            dinvn_sb = small.tile([P, R], F32)
            nc.scalar.activation(dinvn_sb[:], degn_sb[:],
                                 mybir.ActivationFunctionType.Sqrt)
            nc.vector.reciprocal(dinvn_sb[:], dinvn_sb[:])

            # ---- cW = (sum_k emb[k]) @ W  as [1, 10] (fp16)
            csumT_ps = ps0.tile([P, 2], F32, space="PSUM", tag="t0")
            for h in range(2):
                nc.tensor.matmul(csumT_ps[:, h:h + 1],
                                 lhsT=emb_sb[:, h * P:(h + 1) * P],
                                 rhs=ones_col10[:], start=True, stop=True)
            csumT = small.tile([P, 2], F16)
            nc.vector.tensor_copy(csumT[:], csumT_ps[:])
            cw_ps = ps0.tile([1, K], F32, space="PSUM", tag="t0")
            for h in range(2):
                nc.tensor.matmul(cw_ps[:], lhsT=csumT[:, h:h + 1],
                                 rhs=w_sb[:, h * K:(h + 1) * K],
                                 start=(h == 0), stop=(h == 1))
            cw_sb = small.tile([1, K], F16)
            nc.vector.tensor_copy(cw_sb[:], cw_ps[:])

            # bias broadcast to all partitions: [128, 10] f32
            biasb_ps = ps0.tile([P, K], F32, space="PSUM", tag="t0")
            nc.tensor.matmul(biasb_ps[:], lhsT=ones_row_f[:], rhs=bias_sb[:],
                             start=True, stop=True)
            biasb = small.tile([P, K], F32)
            nc.vector.tensor_copy(biasb[:], biasb_ps[:])

            # ---- phase B: g = dinv * (x @ W + cW)   fp16 [128, R*10]
            g_sb = big.tile([P, R * K], F16)
            xT3 = xT[:].rearrange("p (h n) -> p h n", n=NPAD)
            for r in (range(R) if "B" in PHASES else []):
                hw_ps = psHW.tile([P, K], F32, space="PSUM", tag="hw")
                nc.tensor.matmul(hw_ps[:], lhsT=xT3[:, 0, r * P:(r + 1) * P],
                                 rhs=w_sb[:, 0:K], start=True, stop=False)
                nc.tensor.matmul(hw_ps[:], lhsT=xT3[:, 1, r * P:(r + 1) * P],
                                 rhs=w_sb[:, K:2 * K], start=False, stop=False)
                nc.tensor.matmul(hw_ps[:], lhsT=ones_row_h[:], rhs=cw_sb[:],
                                 start=False, stop=True)
                nc.vector.tensor_scalar_mul(
                    g_sb[:, r * K:(r + 1) * K], hw_ps[:], dinvn_sb[:, r:r + 1])

            # ---- packed bounce + all-gather (fp16, 2MB total)
            g_bounce = dram.tile([NPAD, K], F16)
            if "AG" in PHASES:
              nc.sync.dma_start(
                g_bounce[:].rearrange("(r p) j -> p r j", p=P),
                g_sb[:].rearrange("p (r j) -> p r j", j=K))
            g_all = dram.tile([NCORES * NPAD, K], F16, addr_space="Shared")
            if "AG" in PHASES:
              nc.gpsimd.collective_compute(
                "AllGather", mybir.AluOpType.bypass,
                replica_groups=[list(range(NCORES))],
                ins=[g_bounce[:].opt()],
                outs=[g_all[:].opt()],
            )

            # ---- expand packed g into 4 strided window tables
            zrow = small.tile([1, 4 * K], F16)
            nc.vector.memset(zrow[:], 0.0)
            tables = []
            g_all3 = g_all[:].rearrange("(c u) j -> c u j", c=NCORES)
            for w in range(NWIN):
                tb = dram.tile([TROWS, P], F16, tag=f"tbl{w}")
                tables.append(tb)
                if "AG" not in PHASES:
                    continue
                nc.sync.dma_start(
                    tb[0:WINROWS, 0:K].rearrange("(c q) j -> c q j", c=NCORES),
                    g_all3[:, w * QW:(w + 1) * QW, :])
                nc.sync.dma_start(tb[WINROWS:TROWS, 0:K],
                                  zrow[:, w * K:(w + 1) * K])

            # ---- per-window resident idx streams (wrapped int16)
            maxT16 = max(max(T16s), 1)
            winA = big.tile([P, maxT16], I16, tag="winA")
            winB = big.tile([P, maxT16], I16, tag="winB")
            for b0 in range(32, P, 32):
                nc.vector.memset(winA[b0:b0 + 32, :], 0)
                nc.vector.memset(winB[b0:b0 + 32, :], 0)

            # ---- windowed gathers + per-(window,row) partial reduces
            partials = []
            for w in range(NWIN):
                pt = big.tile([P, R * K], F32, tag=f"part{w}")
                nc.vector.memset(pt[:], 0.0)
                partials.append(pt)
            agg_sb = big.tile([P, R * K], F32)

            if "GATHER" in PHASES:
                loaded_w = set()
                for (w, r0, nrows, kk, col0) in chunks:
                    if w not in loaded_w:
                        loaded_w.add(w)
                        wt = winA if w % 2 == 0 else winB
                        t16 = T16s[w]
                        nc.sync.dma_start(wt[0:16, 0:t16],
                                          idx_in[:, wtok0[w] // 16:
                                                 wtok0[w] // 16 + t16])
                        nc.sync.dma_start(wt[16:32, 0:t16],
                                          idx_in[:, wtok0[w] // 16:
                                                 wtok0[w] // 16 + t16])
                    wt = winA if w % 2 == 0 else winB
                    cols = nrows * kk
                    n = cols * P
                    lc0 = col0 * P // 16 - wtok0[w] // 16
                    msg = msgp.tile([P, GCOLS * K], F16, tag="msgbuf")
                    _raw_dma_gather(
                        nc.gpsimd,
                        msg[:, 0:cols * K].rearrange("p (c j) -> p c j", j=K),
                        tables[w][:, 0:K], wt[:, lc0:lc0 + n // 16], n, K, P)
                    nc.vector.tensor_reduce(
                        out=partials[w][:, r0 * K:(r0 + nrows) * K].rearrange(
                            "p (n j) -> p n j", j=K),
                        in_=msg[:, 0:cols * K].rearrange(
                            "p (n k j) -> p n j k", j=K, k=kk),
                        axis=mybir.AxisListType.X,
                        op=mybir.AluOpType.add)

            if "GATHER" in PHASES:
              nc.vector.tensor_add(out=partials[0][:], in0=partials[0][:],
                                 in1=partials[1][:])
              nc.vector.tensor_add(out=partials[2][:], in0=partials[2][:],
                                   in1=partials[3][:])
              nc.vector.tensor_add(out=agg_sb[:], in0=partials[0][:],
                                   in1=partials[2][:])

            # ---- logits = dinv*agg + bias ; softmax -> s (fp16)
            lg = big.tile([P, R * K], F32)
            lg3 = lg[:].rearrange("p (r j) -> p r j", j=K)
            if "SOFTMAX" in PHASES:
              nc.vector.tensor_tensor(
                out=lg3,
                in0=agg_sb[:].rearrange("p (r j) -> p r j", j=K),
                in1=dinv_sb[:].unsqueeze(2).to_broadcast([P, R, K]),
                op=mybir.AluOpType.mult)
              nc.vector.tensor_tensor(
                  out=lg3, in0=lg3,
                  in1=biasb[:].unsqueeze(1).to_broadcast([P, R, K]),
                  op=mybir.AluOpType.add)
              nc.scalar.activation(lg[:], lg[:],
                                   mybir.ActivationFunctionType.Exp)
            den = small.tile([P, R], F32)
            s_sb = big.tile([P, R * K], F16)
            if "SOFTMAX" in PHASES:
              nc.vector.tensor_reduce(out=den[:], in_=lg3,
                                      axis=mybir.AxisListType.X,
                                      op=mybir.AluOpType.add)
              nc.vector.reciprocal(den[:], den[:])
              nc.vector.tensor_tensor(
                  out=s_sb[:].rearrange("p (r j) -> p r j", j=K),
                  in0=lg3,
                  in1=den[:].unsqueeze(2).to_broadcast([P, R, K]),
                  op=mybir.AluOpType.mult)

            # ---- un-permute s from sigma order back to natural rank order
            s_nat = big.tile([P, R * K], F16, tag="snat")
            if "D" in PHASES:
                s_tbl = dram.tile([NPAD, P], F16, tag="stbl")
                nc.sync.dma_start(
                    s_tbl[:, 0:K].rearrange("(r p) j -> p r j", p=P),
                    s_sb[:].rearrange("p (r j) -> p r j", j=K))
                uidx_sb = small.tile([P, NPAD // 16], I16)
                nc.sync.dma_start(uidx_sb[0:32, :], uidx_in[:])
                for b0 in range(32, P, 32):
                    nc.vector.memset(uidx_sb[b0:b0 + 32, :], 0)
                _raw_dma_gather(
                    nc.gpsimd,
                    s_nat[:].rearrange("p (c j) -> p c j", j=K),
                    s_tbl[:, 0:K], uidx_sb[:, :], NPAD, K, P)

            # ---- p = s @ emb ; out = x + p  (fp16 out, 7 rows per DMA batch)
            DST = 7
            for r0 in (range(0, R, DST) if "D" in PHASES else []):
                nr = min(DST, R - r0)
                xload = opsp.tile([P, DST * IN_CH], F16, tag="xload")
                nc.sync.dma_start(
                    xload[:, 0:nr * IN_CH].rearrange(
                        "p (i ch) -> p i ch", ch=IN_CH),
                    x_in[r0 * P:(r0 + nr) * P, :].rearrange(
                        "(i p) ch -> p i ch", p=P))
                ostage = opsp2.tile([P, DST * IN_CH], F16, tag="ostage")
                for i in range(nr):
                    r = r0 + i
                    st_ps = psST.tile([K, P], F16, space="PSUM", tag="stps")
                    nc.tensor.transpose(st_ps[:], s_nat[:, r * K:(r + 1) * K],
                                        ident_h[:])
                    st = opsp.tile([K, P], F16, tag="st")
                    nc.vector.tensor_copy(st[:], st_ps[:])
                    p_ps = psP.tile([P, IN_CH], F32, space="PSUM", tag="pps")
                    nc.tensor.matmul(p_ps[:], lhsT=st[:], rhs=emb_sb[:],
                                     start=True, stop=True)
                    nc.vector.tensor_add(
                        out=ostage[:, i * IN_CH:(i + 1) * IN_CH],
                        in0=p_ps[:],
                        in1=xload[:, i * IN_CH:(i + 1) * IN_CH])
                nc.sync.dma_start(
                    out[r0 * P:(r0 + nr) * P, :].rearrange(
                        "(i p) ch -> p i ch", p=P),
                    ostage[:, 0:nr * IN_CH].rearrange(
                        "p (i ch) -> p i ch", ch=IN_CH))

    nc.compile()
    _BUILD_CACHE[key] = nc
    return nc


# ----------------------------------------------------------------------------
# Entry point
# ----------------------------------------------------------------------------

def kernel(x, edge_index, batch, W, bias, cluster_emb):
    x = np.asarray(x)
    W16 = np.asarray(W, dtype=np.float16)
    bias = np.asarray(bias, dtype=np.float32).reshape(1, K)
    emb16 = np.asarray(cluster_emb, dtype=np.float16)

    plan = host_prep_cached(edge_index)
    nc = build_kernel(plan["K_w"], plan["TOT16"])

    x16 = np.zeros((NCORES, NPAD, IN_CH), dtype=np.float16)
    x16[:, :NPC] = x.reshape(NCORES, NPC, IN_CH)

    in_maps = []
    for c in range(NCORES):
        in_maps.append({
            "x": x16[c],
            "w": W16,
            "bias": bias,
            "emb": emb16,
            "deg": plan["deg"][c],
            "degn": plan["deg_nat"][c],
            "idx": plan["idx"][c],
            "uidx": plan["uidx"][c],
        })

    res = run_bass_kernel_spmd(nc, in_maps, core_ids=list(range(NCORES)))

    out = np.empty((N_NODES, IN_CH), dtype=np.float32)
    for c in range(NCORES):
        out[c * NPC:(c + 1) * NPC] = res.results[c]["out"][:NPC]
    return out


# revision 6
# speedup vs baseline: 1.0732x; 1.0732x over previous
"""Trainium2 Bass kernel for nn_DiffPoolPrompt (GCN conv + softmax pooling prompt).

Reference computation:
    h = x + sum(cluster_emb, 0)
    logits = GCNConv(h, W, bias, edge_index)   # sym-normalized, self-loops
    s = softmax(logits, axis=1)
    out = x + s @ cluster_emb

Distribution (8 NeuronCores), final:
  - Nodes sharded contiguously: core c owns nodes [c*12500, (c+1)*12500);
    x ships fp16 in natural order (no host permutes of x or out).
  - x is transposed once on-device via the XBAR DMA transpose; each core
    computes g = dinv * (x@W + cW) in fp16 (3 matmuls/row-block, f32 PSUM),
    writes a packed [12544, 10] fp16 bounce, AllGathers it (2MB total), then
    expands into a 256B-strided gather table partitioned into 4 windows
    (int16 idx limit).  Window w holds natural ranks [w*3136, (w+1)*3136) of
    every core plus one forced-zero row for padding slots.
  - Edges partitioned by destination core.  Destination slots use a per-core
    degree-sort permutation (sigma) so rank-rows are degree-homogeneous,
    minimizing rectangular-bucket padding.  Uniform-slot-count gather chunks
    (SLACK-bounded greedy row groups) let ONE dma_gather + ONE 4D strided
    reduce per chunk do the segment-sum on-chip.  NOTE: each window's idx
    stream must be DMA'd into its SBUF tile immediately before that window's
    gathers (program order defines the RAW dependency when win tiles rotate).
  - softmax(logits) runs in sigma layout; s is un-permuted back to natural
    order with one small on-device gather through a local strided table.
  - p = s @ emb per row-block; out = x + p written fp16, 7 row-blocks per
    DMA; host converts to f32.  dinv is shipped in BOTH natural order (for
    g) and sigma order (for logits).
"""

import hashlib
import numpy as np

import concourse.bass as bass
import concourse.bacc as bacc
import concourse.tile as tile
import concourse.mybir as mybir
from concourse.bass_utils import run_bass_kernel_spmd
from concourse.masks import make_identity
from concourse import ap_utils

N_NODES = 100000
N_EDGES = 3200000
IN_CH = 256
K = 10
NCORES = 8
P = 128
NPC = N_NODES // NCORES          # 12500 nodes per core
R = (NPC + P - 1) // P           # 98 rank-rows per core
NPAD = P * R                     # 12544 padded ranks per core
NWIN = 4                         # gather windows (int16 idx limit)
WB = [0, 3200, 6400, 9600, NPAD]     # window rank boundaries (row-aligned)
QWS = [WB[i + 1] - WB[i] for i in range(NWIN)]
RB = [b // P for b in WB]            # row-block boundaries
ZPADW = [NCORES * q for q in QWS]    # per-window forced-zero row index
GCOLS = 120                      # gather chunk: 120 cols * 128 = 15360 tokens

F32 = mybir.dt.float32
F16 = mybir.dt.float16
I16 = mybir.dt.int16


def _raw_dma_gather(gp, out_ap, in_ap, idxs_ap, num_idxs, elem_size, elem_step,
                    queue_num=0):
    """bass.dma_gather minus the 256B-elem assert (non-transpose, DRAM src)."""
    assert idxs_ap.dtype == mybir.dt.int16
    assert in_ap.space == bass.MemorySpace.DRAM
    assert idxs_ap.space == bass.MemorySpace.SBUF
    assert out_ap.space == bass.MemorySpace.SBUF
    assert in_ap.dtype == out_ap.dtype
    assert ap_utils.ap_is_contiguous(in_ap.ap[1:])
    assert ap_utils.ap_is_contiguous(out_ap.ap[1:])
    assert ap_utils.ap_is_contiguous(idxs_ap.ap[1:])
    assert in_ap.ap[-1][1] == elem_size and out_ap.ap[-1][1] == elem_size
    assert in_ap.ap[0][0] == elem_step
    stride_bytes = elem_step * mybir.dt.size(in_ap.dtype)
    stride_bytes_256 = stride_bytes // 256
    assert stride_bytes_256 * 256 == stride_bytes and 0 < stride_bytes_256 < 256
    _in_ap = gp.lower_ap_dma(in_ap, for_custom_bir_dma=True)
    _idxs_ap = gp.lower_ap(idxs_ap)
    _out_ap = gp.lower_ap(out_ap)
    return gp.add_instruction(
        mybir.InstDMAGatherAnt(
            name=gp.bass.get_next_instruction_name(),
            ins=[*_in_ap, _idxs_ap, gp.lower_val_access(gp.to_reg(num_idxs))],
            outs=[_out_ap],
            transpose=False,
            num_idxs=num_idxs,
            elem_size=elem_size,
            stride_bytes_256=stride_bytes_256,
            gen_mode=0,
            single_packet=False,
            queue_num=queue_num,
            sbuf_tokens_per_rank=0,
            sbuf_free_dim_per_rank=0,
            sbuf_free_dim_pad_per_rank=0,
            sbuf_byte_offset=0,
        )
    )


# ----------------------------------------------------------------------------
# Host-side sharding / index prep (numpy, index-only)
# ----------------------------------------------------------------------------

def plan_chunks(K_w):
    """Greedy row grouping per window with uniform slot count kk per chunk.
    Returns (chunks, row_coloff, total_cols):
      chunks: list of (w, r0, nrows, kk, col0)
      row_coloff[w, r]: global column offset of row r's slots (-1 if empty)
    """
    K_w = np.asarray(K_w, dtype=np.int64)
    chunks = []
    row_coloff = np.full((NWIN, R), -1, dtype=np.int64)
    col = 0
    for w in range(NWIN):
        r = 0
        while r < R:
            if int(K_w[w, r]) == 0:
                r += 1
                continue
            SLACK = 8
            kk = 0
            nrows = 0
            r2 = r
            while r2 < R:
                kk2 = max(kk, int(K_w[w, r2]))
                extra = (r2 - r + 1) * kk2 - (r2 - r) * kk - int(K_w[w, r2])
                if (r2 - r + 1) * kk2 > GCOLS or (nrows > 0 and extra > SLACK):
                    break
                kk = kk2
                nrows = r2 - r + 1
                r2 += 1
            chunks.append((w, r, nrows, kk, col))
            for i in range(nrows):
                row_coloff[w, r + i] = col + i * kk
            col += nrows * kk
            r += nrows
    return chunks, row_coloff, col


def host_prep(edge_index):
    """Partition + sort edges, build per-core windowed gather index streams."""
    src = np.asarray(edge_index[0], dtype=np.int64)
    dst = np.asarray(edge_index[1], dtype=np.int64)

    deg_all = np.bincount(dst, minlength=N_NODES).astype(np.float32) + 1.0

    # append self-loops
    loop = np.arange(N_NODES, dtype=np.int64)
    src = np.concatenate([src, loop])
    dst = np.concatenate([dst, loop])

    # per-core degree-sort permutation: sigma rank of node v (dst side only)
    rank_sigma = np.empty(N_NODES, dtype=np.int64)
    sig_orders = []
    for cc in range(NCORES):
        ds = deg_all[cc * NPC:(cc + 1) * NPC]
        order = np.argsort(-ds, kind="stable")
        sig_orders.append(order)
        rank_sigma[cc * NPC + order] = np.arange(NPC)

    # dst decomposition: core, partition, row (in sigma space)
    c = dst // NPC
    sr = rank_sigma[dst]
    p = sr % P
    r = sr // P
    # src decomposition: window + within-window table row
    cs = src // NPC
    us = src - cs * NPC
    w = np.minimum(us // 3200, NWIN - 1)
    qws = np.asarray(QWS, dtype=np.int64)
    wb = np.asarray(WB[:NWIN], dtype=np.int64)
    twr = (cs * qws[w] + us - wb[w]).astype(np.int16)

    key = ((c * NWIN + w) * R + r) * P + p
    order = np.argsort(key, kind="stable")
    key_s = key[order]
    twr_s = twr[order]

    cnt = np.bincount(key_s, minlength=NCORES * NWIN * R * P)
    cnt4 = cnt.reshape(NCORES, NWIN, R, P)
    K_w = cnt4.max(axis=(0, 3)).astype(np.int64)          # [NWIN, R]
    assert K_w.max() <= GCOLS, f"bucket overflow: {K_w.max()} > {GCOLS}"

    chunks, row_coloff, total_cols = plan_chunks(K_w)
    TOT = total_cols * P
    TOT16 = TOT // 16
    # window w's column span (chunks of a window are contiguous)
    wcol0 = np.zeros(NWIN + 1, dtype=np.int64)
    for (w, r0, nrows, kk, col0) in chunks:
        wcol0[w + 1] = max(wcol0[w + 1], col0 + nrows * kk)
    for w in range(NWIN):
        wcol0[w + 1] = max(wcol0[w + 1], wcol0[w])

    # within-bucket slot number for every (sorted) edge
    starts = np.concatenate([[0], np.cumsum(cnt)])[:-1]
    k_within = np.arange(key_s.shape[0], dtype=np.int64) - starts[key_s]

    # token position inside the full stream (per core)
    wk = (key_s // (R * P)) % NWIN
    rk = (key_s // P) % R
    pk = key_s % P
    t = (row_coloff[wk, rk] + k_within) * P + pk

    core_of = key_s // (NWIN * R * P)
    core_starts = np.searchsorted(core_of, np.arange(NCORES + 1))

    idxs = []
    degs = []
    degs_nat = []
    uidxs = []
    for cc in range(NCORES):
        lo, hi = core_starts[cc], core_starts[cc + 1]
        flat = np.empty(TOT, dtype=np.int16)
        for w_i in range(NWIN):
            flat[wcol0[w_i] * P:wcol0[w_i + 1] * P] = ZPADW[w_i]
        flat[t[lo:hi]] = twr_s[lo:hi]
        wrap = flat.reshape(TOT16, 16).T                   # [16, TOT16]
        idxs.append(np.ascontiguousarray(wrap))

        dg = np.ones(NPAD, dtype=np.float32)
        dg[:NPC] = deg_all[cc * NPC:(cc + 1) * NPC][sig_orders[cc]]
        degs.append(np.ascontiguousarray(dg.reshape(R, P).T))  # [P, R] sigma
        dn = np.ones(NPAD, dtype=np.float32)
        dn[:NPC] = deg_all[cc * NPC:(cc + 1) * NPC]
        degs_nat.append(np.ascontiguousarray(dn.reshape(R, P).T))

        # un-permute gather stream: natural rank u reads sigma-rank row of u
        flat_u = np.arange(NPAD, dtype=np.int16)
        flat_u[:NPC] = rank_sigma[cc * NPC:(cc + 1) * NPC]
        wrap_u = flat_u.reshape(NPAD // 16, 16).T
        uidxs.append(np.ascontiguousarray(
            np.concatenate([wrap_u, wrap_u], axis=0)))     # [32, 784]

    return {"deg": degs, "deg_nat": degs_nat, "idx": idxs, "uidx": uidxs,
            "K_w": K_w, "TOT16": TOT16}


_PREP_CACHE = {}


def host_prep_cached(edge_index):
    e = np.asarray(edge_index)
    samp = e[:, ::997].tobytes()
    fp = (e.shape, str(e.dtype), int(e.sum()),
          hashlib.blake2b(samp, digest_size=16).hexdigest())
    hit = _PREP_CACHE.get(fp)
    if hit is None:
        hit = host_prep(e)
        _PREP_CACHE.clear()
        _PREP_CACHE[fp] = hit
    return hit


# ----------------------------------------------------------------------------
# Device kernel
# ----------------------------------------------------------------------------

_BUILD_CACHE = {}

# experiment knobs (overridden by prof scripts; defaults = production)
NQUEUES = 1
PHASES = frozenset({"B", "AG", "GATHER", "SOFTMAX", "D"})


def build_kernel(K_w, TOT16):
    K_w = np.asarray(K_w, dtype=np.int64)
    key = (TOT16, NQUEUES, PHASES) + tuple(int(k) for k in K_w.ravel())
    if key in _BUILD_CACHE:
        return _BUILD_CACHE[key]
    chunks, row_coloff, total_cols = plan_chunks(K_w)
    assert total_cols * P == TOT16 * 16
    wtok0 = [None] * NWIN
    wtokend = [0] * NWIN
    for (w, r0, nrows, kk, col0) in chunks:
        if wtok0[w] is None:
            wtok0[w] = col0 * P
        wtokend[w] = (col0 + nrows * kk) * P
    T16s = [(wtokend[w] - (wtok0[w] or 0)) // 16 for w in range(NWIN)]

    nc = bacc.Bacc("TRN2", target_bir_lowering=False, debug=False,
                   num_devices=NCORES, num_swdge_queues=NQUEUES)

    x_in = nc.dram_tensor("x", [NPAD, IN_CH], F16, kind="ExternalInput").ap()
    w_in = nc.dram_tensor("w", [IN_CH, K], F16, kind="ExternalInput").ap()
    bias_in = nc.dram_tensor("bias", [1, K], F32, kind="ExternalInput").ap()
    emb_in = nc.dram_tensor("emb", [K, IN_CH], F16, kind="ExternalInput").ap()
    deg_in = nc.dram_tensor("deg", [P, R], F32, kind="ExternalInput").ap()
    degn_in = nc.dram_tensor("degn", [P, R], F32, kind="ExternalInput").ap()
    idx_in = nc.dram_tensor("idx", [16, TOT16], I16, kind="ExternalInput").ap()
    uidx_in = nc.dram_tensor("uidx", [32, NPAD // 16], I16,
                             kind="ExternalInput").ap()
    out = nc.dram_tensor("out", [NPAD, IN_CH], F16, kind="ExternalOutput").ap()

    with tile.TileContext(nc) as tc:
        with tc.tile_pool(name="big", bufs=1) as big, \
             tc.tile_pool(name="small", bufs=1) as small, \
             tc.tile_pool(name="xt", bufs=4) as xtp, \
             tc.tile_pool(name="msg", bufs=6) as msgp, \
             tc.tile_pool(name="ops", bufs=3) as opsp, \
             tc.tile_pool(name="ops2", bufs=3) as opsp2, \
             tc.tile_pool(name="ps0", bufs=1, space="PSUM") as ps0, \
             tc.tile_pool(name="psHW", bufs=3, space="PSUM") as psHW, \
             tc.tile_pool(name="psST", bufs=2, space="PSUM") as psST, \
             tc.tile_pool(name="psP", bufs=2, space="PSUM") as psP, \
             tc.tile_pool(name="dram", bufs=1, space="DRAM") as dram:

            # ---- resident loads: x transposed once via the XBAR
            xT = big.tile([P, 2 * NPAD], F16)             # [ch%128, ch//128, n]
            for q in range(NWIN):
                nc.sync.dma_start_transpose(
                    xT[:].rearrange("p (h n) -> p h n", n=NPAD)
                        [:, :, WB[q]:WB[q + 1]],
                    x_in[WB[q]:WB[q + 1], :])
            w_sb = small.tile([P, 2 * K], F16)            # [ch%128, 2 chunks]
            nc.sync.dma_start(w_sb[:, 0:K], w_in[0:P, :])
            nc.sync.dma_start(w_sb[:, K:2 * K], w_in[P:2 * P, :])
            emb_sb = small.tile([K, IN_CH], F16)
            nc.sync.dma_start(emb_sb[:], emb_in[:])
            deg_sb = small.tile([P, R], F32)
            nc.sync.dma_start(deg_sb[:], deg_in[:])
            degn_sb = small.tile([P, R], F32)
            nc.sync.dma_start(degn_sb[:], degn_in[:])
            bias_sb = small.tile([1, K], F32)
            nc.sync.dma_start(bias_sb[:], bias_in[:])

            ident_h = small.tile([P, P], F16)
            make_identity(nc, ident_h[:])

            ones_row_h = small.tile([1, P], F16)
            nc.vector.memset(ones_row_h[:], 1.0)
            ones_row_f = small.tile([1, P], F32)
            nc.vector.memset(ones_row_f[:], 1.0)
            ones_col10 = small.tile([K, 1], F16)
            nc.vector.memset(ones_col10[:], 1.0)

            # ---- dinv = 1/sqrt(deg): sigma order (logits) + natural (g)
            dinv_sb = small.tile([P, R], F32)
            nc.scalar.activation(dinv_sb[:], deg_sb[:],
                                 mybir.ActivationFunctionType.Sqrt)
            nc.vector.reciprocal(dinv_sb[:], dinv_sb[:])
            dinvn_sb = small.tile([P, R], F32)
            nc.scalar.activation(dinvn_sb[:], degn_sb[:],
                                 mybir.ActivationFunctionType.Sqrt)
            nc.vector.reciprocal(dinvn_sb[:], dinvn_sb[:])

            # ---- cW = (sum_k emb[k]) @ W  as [1, 10] (fp16)
            csumT_ps = ps0.tile([P, 2], F32, space="PSUM", tag="t0")
            for h in range(2):
                nc.tensor.matmul(csumT_ps[:, h:h + 1],
                                 lhsT=emb_sb[:, h * P:(h + 1) * P],
                                 rhs=ones_col10[:], start=True, stop=True)
            csumT = small.tile([P, 2], F16)
            nc.vector.tensor_copy(csumT[:], csumT_ps[:])
            cw_ps = ps0.tile([1, K], F32, space="PSUM", tag="t0")
            for h in range(2):
                nc.tensor.matmul(cw_ps[:], lhsT=csumT[:, h:h + 1],
                                 rhs=w_sb[:, h * K:(h + 1) * K],
                                 start=(h == 0), stop=(h == 1))
            cw_sb = small.tile([1, K], F16)
            nc.vector.tensor_copy(cw_sb[:], cw_ps[:])

            # bias broadcast to all partitions: [128, 10] f32
            biasb_ps = ps0.tile([P, K], F32, space="PSUM", tag="t0")
            nc.tensor.matmul(biasb_ps[:], lhsT=ones_row_f[:], rhs=bias_sb[:],
                             start=True, stop=True)
            biasb = small.tile([P, K], F32)
            nc.vector.tensor_copy(biasb[:], biasb_ps[:])

            # ---- phase B: g = dinv * (x @ W + cW)   fp16 [128, R*10]
            g_sb = big.tile([P, R * K], F16)
            xT3 = xT[:].rearrange("p (h n) -> p h n", n=NPAD)
            for r in (range(R) if "B" in PHASES else []):
                hw_ps = psHW.tile([P, K], F32, space="PSUM", tag="hw")
                nc.tensor.matmul(hw_ps[:], lhsT=xT3[:, 0, r * P:(r + 1) * P],
                                 rhs=w_sb[:, 0:K], start=True, stop=False)
                nc.tensor.matmul(hw_ps[:], lhsT=xT3[:, 1, r * P:(r + 1) * P],
                                 rhs=w_sb[:, K:2 * K], start=False, stop=False)
                nc.tensor.matmul(hw_ps[:], lhsT=ones_row_h[:], rhs=cw_sb[:],
                                 start=False, stop=True)
                nc.vector.tensor_scalar_mul(
                    g_sb[:, r * K:(r + 1) * K], hw_ps[:], dinvn_sb[:, r:r + 1])

            # ---- packed bounce + all-gather (fp16, 2MB total), per window
            g_bounce = dram.tile([NPAD, K], F16)
            if "AG" in PHASES:
              for w in range(NWIN):
                nc.sync.dma_start(
                    g_bounce[WB[w]:WB[w + 1], :].rearrange(
                        "(r p) j -> p r j", p=P),
                    g_sb[:, RB[w] * K:RB[w + 1] * K].rearrange(
                        "p (r j) -> p r j", j=K))
            # ---- per-window all-gather + expand into strided tables
            zrow = small.tile([1, 4 * K], F16)
            nc.vector.memset(zrow[:], 0.0)
            tables = []
            for w in range(NWIN):
                wrows = NCORES * QWS[w]
                tb = dram.tile([wrows + 1, P], F16, tag=f"tbl{w}")
                tables.append(tb)
                ga = dram.tile([wrows, K], F16, addr_space="Shared",
                               tag=f"gall{w}")
                if "AG" not in PHASES:
                    continue
                nc.gpsimd.collective_compute(
                    "AllGather", mybir.AluOpType.bypass,
                    replica_groups=[list(range(NCORES))],
                    ins=[g_bounce[WB[w]:WB[w + 1], :].opt()],
                    outs=[ga[:].opt()],
                )
                nc.sync.dma_start(tb[0:wrows, 0:K], ga[:])
                nc.sync.dma_start(tb[wrows:wrows + 1, 0:K],
                                  zrow[:, w * K:(w + 1) * K])

            # ---- per-window resident idx streams (wrapped int16)
            maxT16 = max(max(T16s), 1)
            winA = big.tile([P, maxT16], I16, tag="winA")
            winB = big.tile([P, maxT16], I16, tag="winB")
            for b0 in range(32, P, 32):
                nc.vector.memset(winA[b0:b0 + 32, :], 0)
                nc.vector.memset(winB[b0:b0 + 32, :], 0)

            # ---- windowed gathers + per-(window,row) partial reduces
            partials = []
            for w in range(NWIN):
                pt = big.tile([P, R * K], F32, tag=f"part{w}")
                nc.vector.memset(pt[:], 0.0)
                partials.append(pt)
            agg_sb = big.tile([P, R * K], F32)

            if "GATHER" in PHASES:
                loaded_w = set()
                for (w, r0, nrows, kk, col0) in chunks:
                    if w not in loaded_w:
                        loaded_w.add(w)
                        wt = winA if w % 2 == 0 else winB
                        t16 = T16s[w]
                        nc.sync.dma_start(wt[0:16, 0:t16],
                                          idx_in[:, wtok0[w] // 16:
                                                 wtok0[w] // 16 + t16])
                        nc.sync.dma_start(wt[16:32, 0:t16],
                                          idx_in[:, wtok0[w] // 16:
                                                 wtok0[w] // 16 + t16])
                    wt = winA if w % 2 == 0 else winB
                    cols = nrows * kk
                    n = cols * P
                    lc0 = col0 * P // 16 - wtok0[w] // 16
                    msg = msgp.tile([P, GCOLS * K], F16, tag="msgbuf")
                    _raw_dma_gather(
                        nc.gpsimd,
                        msg[:, 0:cols * K].rearrange("p (c j) -> p c j", j=K),
                        tables[w][:, 0:K], wt[:, lc0:lc0 + n // 16], n, K, P)
                    nc.vector.tensor_reduce(
                        out=partials[w][:, r0 * K:(r0 + nrows) * K].rearrange(
                            "p (n j) -> p n j", j=K),
                        in_=msg[:, 0:cols * K].rearrange(
                            "p (n k j) -> p n j k", j=K, k=kk),
                        axis=mybir.AxisListType.X,
                        op=mybir.AluOpType.add)

            if "GATHER" in PHASES:
              nc.vector.tensor_add(out=partials[0][:], in0=partials[0][:],
                                 in1=partials[1][:])
              nc.vector.tensor_add(out=partials[2][:], in0=partials[2][:],
                                   in1=partials[3][:])
              nc.vector.tensor_add(out=agg_sb[:], in0=partials[0][:],
                                   in1=partials[2][:])

            # ---- logits = dinv*agg + bias ; softmax -> s (fp16)
            lg = big.tile([P, R * K], F32)
            lg3 = lg[:].rearrange("p (r j) -> p r j", j=K)
            if "SOFTMAX" in PHASES:
              nc.vector.tensor_tensor(
                out=lg3,
                in0=agg_sb[:].rearrange("p (r j) -> p r j", j=K),
                in1=dinv_sb[:].unsqueeze(2).to_broadcast([P, R, K]),
                op=mybir.AluOpType.mult)
              nc.vector.tensor_tensor(
                  out=lg3, in0=lg3,
                  in1=biasb[:].unsqueeze(1).to_broadcast([P, R, K]),
                  op=mybir.AluOpType.add)
              nc.scalar.activation(lg[:], lg[:],
                                   mybir.ActivationFunctionType.Exp)
            den = small.tile([P, R], F32)
            s_sb = big.tile([P, R * K], F16)
            if "SOFTMAX" in PHASES:
              nc.vector.tensor_reduce(out=den[:], in_=lg3,
                                      axis=mybir.AxisListType.X,
                                      op=mybir.AluOpType.add)
              nc.vector.reciprocal(den[:], den[:])
              nc.vector.tensor_tensor(
                  out=s_sb[:].rearrange("p (r j) -> p r j", j=K),
                  in0=lg3,
                  in1=den[:].unsqueeze(2).to_broadcast([P, R, K]),
                  op=mybir.AluOpType.mult)

            # ---- un-permute s from sigma order back to natural rank order
            s_nat = big.tile([P, R * K], F16, tag="snat")
            if "D" in PHASES:
                s_tbl = dram.tile([NPAD, P], F16, tag="stbl")
                nc.sync.dma_start(
                    s_tbl[:, 0:K].rearrange("(r p) j -> p r j", p=P),
                    s_sb[:].rearrange("p (r j) -> p r j", j=K))
                uidx_sb = small.tile([P, NPAD // 16], I16)
                nc.sync.dma_start(uidx_sb[0:32, :], uidx_in[:])
                for b0 in range(32, P, 32):
                    nc.vector.memset(uidx_sb[b0:b0 + 32, :], 0)
                _raw_dma_gather(
                    nc.gpsimd,
                    s_nat[:].rearrange("p (c j) -> p c j", j=K),
                    s_tbl[:, 0:K], uidx_sb[:, :], NPAD, K, P)

            # ---- p = s @ emb ; out = x + p  (fp16 out, 7 rows per DMA batch)
            DST = 7
            for r0 in (range(0, R, DST) if "D" in PHASES else []):
                nr = min(DST, R - r0)
                xload = opsp.tile([P, DST * IN_CH], F16, tag="xload")
                nc.sync.dma_start(
                    xload[:, 0:nr * IN_CH].rearrange(
                        "p (i ch) -> p i ch", ch=IN_CH),
                    x_in[r0 * P:(r0 + nr) * P, :].rearrange(
                        "(i p) ch -> p i ch", p=P))
                ostage = opsp2.tile([P, DST * IN_CH], F16, tag="ostage")
                for i in range(nr):
                    r = r0 + i
                    st_ps = psST.tile([K, P], F16, space="PSUM", tag="stps")
                    nc.tensor.transpose(st_ps[:], s_nat[:, r * K:(r + 1) * K],
                                        ident_h[:])
                    st = opsp.tile([K, P], F16, tag="st")
                    nc.vector.tensor_copy(st[:], st_ps[:])
                    p_ps = psP.tile([P, IN_CH], F32, space="PSUM", tag="pps")
                    nc.tensor.matmul(p_ps[:], lhsT=st[:], rhs=emb_sb[:],
                                     start=True, stop=True)
                    nc.vector.tensor_add(
                        out=ostage[:, i * IN_CH:(i + 1) * IN_CH],
                        in0=p_ps[:],
                        in1=xload[:, i * IN_CH:(i + 1) * IN_CH])
                nc.sync.dma_start(
                    out[r0 * P:(r0 + nr) * P, :].rearrange(
                        "(i p) ch -> p i ch", p=P),
                    ostage[:, 0:nr * IN_CH].rearrange(
                        "p (i ch) -> p i ch", ch=IN_CH))

    nc.compile()
    _BUILD_CACHE[key] = nc
    return nc


# ----------------------------------------------------------------------------
# Entry point
# ----------------------------------------------------------------------------

def kernel(x, edge_index, batch, W, bias, cluster_emb):
    x = np.asarray(x)
    W16 = np.asarray(W, dtype=np.float16)
    bias = np.asarray(bias, dtype=np.float32).reshape(1, K)
    emb16 = np.asarray(cluster_emb, dtype=np.float16)

    plan = host_prep_cached(edge_index)
    nc = build_kernel(plan["K_w"], plan["TOT16"])

    x16 = np.zeros((NCORES, NPAD, IN_CH), dtype=np.float16)
    x16[:, :NPC] = x.reshape(NCORES, NPC, IN_CH)

    in_maps = []
    for c in range(NCORES):
        in_maps.append({
            "x": x16[c],
            "w": W16,
            "bias": bias,
            "emb": emb16,
            "deg": plan["deg"][c],
            "degn": plan["deg_nat"][c],
            "idx": plan["idx"][c],
            "uidx": plan["uidx"][c],
        })

    res = run_bass_kernel_spmd(nc, in_maps, core_ids=list(range(NCORES)))

    out = np.empty((N_NODES, IN_CH), dtype=np.float32)
    for c in range(NCORES):
        out[c * NPC:(c + 1) * NPC] = res.results[c]["out"][:NPC]
    return out


# revision 7
# speedup vs baseline: 1.1299x; 1.0528x over previous
"""Trainium2 Bass kernel for nn_DiffPoolPrompt (GCN conv + softmax pooling prompt).

Reference computation:
    h = x + sum(cluster_emb, 0)
    logits = GCNConv(h, W, bias, edge_index)   # sym-normalized, self-loops
    s = softmax(logits, axis=1)
    out = x + s @ cluster_emb

Distribution (8 NeuronCores), final:
  - Nodes sharded contiguously: core c owns nodes [c*12500, (c+1)*12500);
    x ships fp16 in natural order (no host permutes of x or out).
  - x is transposed once on-device via the XBAR DMA transpose; each core
    computes g = dinv * (x@W + cW) in fp16 (3 matmuls/row-block, f32 PSUM),
    writes a packed [12544, 10] fp16 bounce, AllGathers it (2MB total), then
    expands into a 256B-strided gather table partitioned into 4 windows
    (int16 idx limit).  Windows are row-block aligned (3200/3200/3200/2944
    ranks) so each window's bounce + AllGather launches as soon as its
    quarter of phase B completes; each window table carries one forced-zero
    row for padding slots.
  - Edges partitioned by destination core.  Destination slots use a per-core
    degree-sort permutation (sigma) so rank-rows are degree-homogeneous,
    minimizing rectangular-bucket padding.  Rows keep their exact per-row
    slot counts; greedy row groups form one dma_gather per ~120 columns,
    reduced by one 4D strided reduce per run of equal-K consecutive rows.  NOTE: each window's idx
    stream must be DMA'd into its SBUF tile immediately before that window's
    gathers (program order defines the RAW dependency when win tiles rotate).
  - softmax(logits) runs in sigma layout; s is un-permuted back to natural
    order with one small on-device gather through a local strided table.
  - p = s @ emb per row-block; out = x + p written fp16, 7 row-blocks per
    DMA; host converts to f32.  dinv is shipped in BOTH natural order (for
    g) and sigma order (for logits).
"""

import hashlib
import numpy as np

import concourse.bass as bass
import concourse.bacc as bacc
import concourse.tile as tile
import concourse.mybir as mybir
from concourse.bass_utils import run_bass_kernel_spmd
from concourse.masks import make_identity
from concourse import ap_utils

N_NODES = 100000
N_EDGES = 3200000
IN_CH = 256
K = 10
NCORES = 8
P = 128
NPC = N_NODES // NCORES          # 12500 nodes per core
R = (NPC + P - 1) // P           # 98 rank-rows per core
NPAD = P * R                     # 12544 padded ranks per core
NWIN = 4                         # gather windows (int16 idx limit)
WB = [0, 3200, 6400, 9600, NPAD]     # window rank boundaries (row-aligned)
QWS = [WB[i + 1] - WB[i] for i in range(NWIN)]
RB = [b // P for b in WB]            # row-block boundaries
ZPADW = [NCORES * q for q in QWS]    # per-window forced-zero row index
GCOLS = 120                      # gather chunk: 120 cols * 128 = 15360 tokens

F32 = mybir.dt.float32
F16 = mybir.dt.float16
I16 = mybir.dt.int16


def _raw_dma_gather(gp, out_ap, in_ap, idxs_ap, num_idxs, elem_size, elem_step,
                    queue_num=0):
    """bass.dma_gather minus the 256B-elem assert (non-transpose, DRAM src)."""
    assert idxs_ap.dtype == mybir.dt.int16
    assert in_ap.space == bass.MemorySpace.DRAM
    assert idxs_ap.space == bass.MemorySpace.SBUF
    assert out_ap.space == bass.MemorySpace.SBUF
    assert in_ap.dtype == out_ap.dtype
    assert ap_utils.ap_is_contiguous(in_ap.ap[1:])
    assert ap_utils.ap_is_contiguous(out_ap.ap[1:])
    assert ap_utils.ap_is_contiguous(idxs_ap.ap[1:])
    assert in_ap.ap[-1][1] == elem_size and out_ap.ap[-1][1] == elem_size
    assert in_ap.ap[0][0] == elem_step
    stride_bytes = elem_step * mybir.dt.size(in_ap.dtype)
    stride_bytes_256 = stride_bytes // 256
    assert stride_bytes_256 * 256 == stride_bytes and 0 < stride_bytes_256 < 256
    _in_ap = gp.lower_ap_dma(in_ap, for_custom_bir_dma=True)
    _idxs_ap = gp.lower_ap(idxs_ap)
    _out_ap = gp.lower_ap(out_ap)
    return gp.add_instruction(
        mybir.InstDMAGatherAnt(
            name=gp.bass.get_next_instruction_name(),
            ins=[*_in_ap, _idxs_ap, gp.lower_val_access(gp.to_reg(num_idxs))],
            outs=[_out_ap],
            transpose=False,
            num_idxs=num_idxs,
            elem_size=elem_size,
            stride_bytes_256=stride_bytes_256,
            gen_mode=0,
            single_packet=False,
            queue_num=queue_num,
            sbuf_tokens_per_rank=0,
            sbuf_free_dim_per_rank=0,
            sbuf_free_dim_pad_per_rank=0,
            sbuf_byte_offset=0,
        )
    )


# ----------------------------------------------------------------------------
# Host-side sharding / index prep (numpy, index-only)
# ----------------------------------------------------------------------------

def plan_chunks(K_w):
    """Greedy row grouping per window: rows keep their EXACT slot count
    K_w[w,r] (no uniformity padding); each chunk is one dma_gather, reduced
    by runs of consecutive equal-K rows.
    Returns (chunks, row_coloff, total_cols):
      chunks: list of (w, rows_tuple, col0, cols)
      row_coloff[w, r]: global column offset of row r's slots (-1 if empty)
    """
    K_w = np.asarray(K_w, dtype=np.int64)
    chunks = []
    row_coloff = np.full((NWIN, R), -1, dtype=np.int64)
    col = 0
    for w in range(NWIN):
        r = 0
        while r < R:
            if int(K_w[w, r]) == 0:
                r += 1
                continue
            rows = []
            cols = 0
            r2 = r
            while r2 < R and cols + int(K_w[w, r2]) <= GCOLS:
                if int(K_w[w, r2]) > 0:
                    rows.append(r2)
                    cols += int(K_w[w, r2])
                r2 += 1
            col0 = col
            for rr in rows:
                row_coloff[w, rr] = col
                col += int(K_w[w, rr])
            chunks.append((w, tuple(rows), col0, cols))
            r = r2
    return chunks, row_coloff, col


def host_prep(edge_index):
    """Partition + sort edges, build per-core windowed gather index streams."""
    src = np.asarray(edge_index[0], dtype=np.int64)
    dst = np.asarray(edge_index[1], dtype=np.int64)

    deg_all = np.bincount(dst, minlength=N_NODES).astype(np.float32) + 1.0

    # append self-loops
    loop = np.arange(N_NODES, dtype=np.int64)
    src = np.concatenate([src, loop])
    dst = np.concatenate([dst, loop])

    # per-core degree-sort permutation: sigma rank of node v (dst side only)
    rank_sigma = np.empty(N_NODES, dtype=np.int64)
    sig_orders = []
    for cc in range(NCORES):
        ds = deg_all[cc * NPC:(cc + 1) * NPC]
        order = np.argsort(-ds, kind="stable")
        sig_orders.append(order)
        rank_sigma[cc * NPC + order] = np.arange(NPC)

    # dst decomposition: core, partition, row (in sigma space)
    c = dst // NPC
    sr = rank_sigma[dst]
    p = sr % P
    r = sr // P
    # src decomposition: window + within-window table row
    cs = src // NPC
    us = src - cs * NPC
    w = np.minimum(us // 3200, NWIN - 1)
    qws = np.asarray(QWS, dtype=np.int64)
    wb = np.asarray(WB[:NWIN], dtype=np.int64)
    twr = (cs * qws[w] + us - wb[w]).astype(np.int16)

    key = ((c * NWIN + w) * R + r) * P + p
    order = np.argsort(key, kind="stable")
    key_s = key[order]
    twr_s = twr[order]

    cnt = np.bincount(key_s, minlength=NCORES * NWIN * R * P)
    cnt4 = cnt.reshape(NCORES, NWIN, R, P)
    K_w = cnt4.max(axis=(0, 3)).astype(np.int64)          # [NWIN, R]
    assert K_w.max() <= GCOLS, f"bucket overflow: {K_w.max()} > {GCOLS}"

    chunks, row_coloff, total_cols = plan_chunks(K_w)
    TOT = total_cols * P
    TOT16 = TOT // 16
    # window w's column span (chunks of a window are contiguous)
    wcol0 = np.zeros(NWIN + 1, dtype=np.int64)
    for (w, rows, col0, cols) in chunks:
        wcol0[w + 1] = max(wcol0[w + 1], col0 + cols)
    for w in range(NWIN):
        wcol0[w + 1] = max(wcol0[w + 1], wcol0[w])

    # within-bucket slot number for every (sorted) edge
    starts = np.concatenate([[0], np.cumsum(cnt)])[:-1]
    k_within = np.arange(key_s.shape[0], dtype=np.int64) - starts[key_s]

    # token position inside the full stream (per core)
    wk = (key_s // (R * P)) % NWIN
    rk = (key_s // P) % R
    pk = key_s % P
    t = (row_coloff[wk, rk] + k_within) * P + pk

    core_of = key_s // (NWIN * R * P)
    core_starts = np.searchsorted(core_of, np.arange(NCORES + 1))

    idxs = []
    degs = []
    degs_nat = []
    uidxs = []
    for cc in range(NCORES):
        lo, hi = core_starts[cc], core_starts[cc + 1]
        flat = np.empty(TOT, dtype=np.int16)
        for w_i in range(NWIN):
            flat[wcol0[w_i] * P:wcol0[w_i + 1] * P] = ZPADW[w_i]
        flat[t[lo:hi]] = twr_s[lo:hi]
        wrap = flat.reshape(TOT16, 16).T                   # [16, TOT16]
        idxs.append(np.ascontiguousarray(wrap))

        dg = np.ones(NPAD, dtype=np.float32)
        dg[:NPC] = deg_all[cc * NPC:(cc + 1) * NPC][sig_orders[cc]]
        degs.append(np.ascontiguousarray(dg.reshape(R, P).T))  # [P, R] sigma
        dn = np.ones(NPAD, dtype=np.float32)
        dn[:NPC] = deg_all[cc * NPC:(cc + 1) * NPC]
        degs_nat.append(np.ascontiguousarray(dn.reshape(R, P).T))

        # un-permute gather stream: natural rank u reads sigma-rank row of u
        flat_u = np.arange(NPAD, dtype=np.int16)
        flat_u[:NPC] = rank_sigma[cc * NPC:(cc + 1) * NPC]
        wrap_u = flat_u.reshape(NPAD // 16, 16).T
        uidxs.append(np.ascontiguousarray(
            np.concatenate([wrap_u, wrap_u], axis=0)))     # [32, 784]

    return {"deg": degs, "deg_nat": degs_nat, "idx": idxs, "uidx": uidxs,
            "K_w": K_w, "TOT16": TOT16}


_PREP_CACHE = {}


def host_prep_cached(edge_index):
    e = np.asarray(edge_index)
    samp = e[:, ::997].tobytes()
    fp = (e.shape, str(e.dtype), int(e.sum()),
          hashlib.blake2b(samp, digest_size=16).hexdigest())
    hit = _PREP_CACHE.get(fp)
    if hit is None:
        hit = host_prep(e)
        _PREP_CACHE.clear()
        _PREP_CACHE[fp] = hit
    return hit


# ----------------------------------------------------------------------------
# Device kernel
# ----------------------------------------------------------------------------

_BUILD_CACHE = {}

# experiment knobs (overridden by prof scripts; defaults = production)
NQUEUES = 1
PHASES = frozenset({"B", "AG", "GATHER", "SOFTMAX", "D"})


def build_kernel(K_w, TOT16):
    K_w = np.asarray(K_w, dtype=np.int64)
    key = (TOT16, NQUEUES, PHASES) + tuple(int(k) for k in K_w.ravel())
    if key in _BUILD_CACHE:
        return _BUILD_CACHE[key]
    chunks, row_coloff, total_cols = plan_chunks(K_w)
    assert total_cols * P == TOT16 * 16
    wtok0 = [None] * NWIN
    wtokend = [0] * NWIN
    for (w, rows, col0, cols) in chunks:
        if wtok0[w] is None:
            wtok0[w] = col0 * P
        wtokend[w] = (col0 + cols) * P
    T16s = [(wtokend[w] - (wtok0[w] or 0)) // 16 for w in range(NWIN)]

    nc = bacc.Bacc("TRN2", target_bir_lowering=False, debug=False,
                   num_devices=NCORES, num_swdge_queues=NQUEUES)

    x_in = nc.dram_tensor("x", [NPAD, IN_CH], F16, kind="ExternalInput").ap()
    w_in = nc.dram_tensor("w", [IN_CH, K], F16, kind="ExternalInput").ap()
    bias_in = nc.dram_tensor("bias", [1, K], F32, kind="ExternalInput").ap()
    emb_in = nc.dram_tensor("emb", [K, IN_CH], F16, kind="ExternalInput").ap()
    deg_in = nc.dram_tensor("deg", [P, R], F32, kind="ExternalInput").ap()
    degn_in = nc.dram_tensor("degn", [P, R], F32, kind="ExternalInput").ap()
    idx_in = nc.dram_tensor("idx", [16, TOT16], I16, kind="ExternalInput").ap()
    uidx_in = nc.dram_tensor("uidx", [32, NPAD // 16], I16,
                             kind="ExternalInput").ap()
    out = nc.dram_tensor("out", [NPAD, IN_CH], F16, kind="ExternalOutput").ap()

    with tile.TileContext(nc) as tc:
        with tc.tile_pool(name="big", bufs=1) as big, \
             tc.tile_pool(name="small", bufs=1) as small, \
             tc.tile_pool(name="xt", bufs=4) as xtp, \
             tc.tile_pool(name="msg", bufs=6) as msgp, \
             tc.tile_pool(name="ops", bufs=3) as opsp, \
             tc.tile_pool(name="ops2", bufs=3) as opsp2, \
             tc.tile_pool(name="ps0", bufs=1, space="PSUM") as ps0, \
             tc.tile_pool(name="psHW", bufs=3, space="PSUM") as psHW, \
             tc.tile_pool(name="psST", bufs=2, space="PSUM") as psST, \
             tc.tile_pool(name="psP", bufs=2, space="PSUM") as psP, \
             tc.tile_pool(name="dram", bufs=1, space="DRAM") as dram:

            # ---- resident loads: x transposed once via the XBAR
            xT = big.tile([P, 2 * NPAD], F16)             # [ch%128, ch//128, n]
            for q in range(NWIN):
                nc.sync.dma_start_transpose(
                    xT[:].rearrange("p (h n) -> p h n", n=NPAD)
                        [:, :, WB[q]:WB[q + 1]],
                    x_in[WB[q]:WB[q + 1], :])
            w_sb = small.tile([P, 2 * K], F16)            # [ch%128, 2 chunks]
            nc.sync.dma_start(w_sb[:, 0:K], w_in[0:P, :])
            nc.sync.dma_start(w_sb[:, K:2 * K], w_in[P:2 * P, :])
            emb_sb = small.tile([K, IN_CH], F16)
            nc.sync.dma_start(emb_sb[:], emb_in[:])
            deg_sb = small.tile([P, R], F32)
            nc.sync.dma_start(deg_sb[:], deg_in[:])
            degn_sb = small.tile([P, R], F32)
            nc.sync.dma_start(degn_sb[:], degn_in[:])
            bias_sb = small.tile([1, K], F32)
            nc.sync.dma_start(bias_sb[:], bias_in[:])

            ident_h = small.tile([P, P], F16)
            make_identity(nc, ident_h[:])

            ones_row_h = small.tile([1, P], F16)
            nc.vector.memset(ones_row_h[:], 1.0)
            ones_row_f = small.tile([1, P], F32)
            nc.vector.memset(ones_row_f[:], 1.0)
            ones_col10 = small.tile([K, 1], F16)
            nc.vector.memset(ones_col10[:], 1.0)

            # ---- dinv = 1/sqrt(deg): sigma order (logits) + natural (g)
            dinv_sb = small.tile([P, R], F32)
            nc.scalar.activation(dinv_sb[:], deg_sb[:],
                                 mybir.ActivationFunctionType.Sqrt)
            nc.vector.reciprocal(dinv_sb[:], dinv_sb[:])
            dinvn_sb = small.tile([P, R], F32)
            nc.scalar.activation(dinvn_sb[:], degn_sb[:],
                                 mybir.ActivationFunctionType.Sqrt)
            nc.vector.reciprocal(dinvn_sb[:], dinvn_sb[:])

            # ---- cW = (sum_k emb[k]) @ W  as [1, 10] (fp16)
            csumT_ps = ps0.tile([P, 2], F32, space="PSUM", tag="t0")
            for h in range(2):
                nc.tensor.matmul(csumT_ps[:, h:h + 1],
                                 lhsT=emb_sb[:, h * P:(h + 1) * P],
                                 rhs=ones_col10[:], start=True, stop=True)
            csumT = small.tile([P, 2], F16)
            nc.vector.tensor_copy(csumT[:], csumT_ps[:])
            cw_ps = ps0.tile([1, K], F32, space="PSUM", tag="t0")
            for h in range(2):
                nc.tensor.matmul(cw_ps[:], lhsT=csumT[:, h:h + 1],
                                 rhs=w_sb[:, h * K:(h + 1) * K],
                                 start=(h == 0), stop=(h == 1))
            cw_sb = small.tile([1, K], F16)
            nc.vector.tensor_copy(cw_sb[:], cw_ps[:])

            # bias broadcast to all partitions: [128, 10] f32
            biasb_ps = ps0.tile([P, K], F32, space="PSUM", tag="t0")
            nc.tensor.matmul(biasb_ps[:], lhsT=ones_row_f[:], rhs=bias_sb[:],
                             start=True, stop=True)
            biasb = small.tile([P, K], F32)
            nc.vector.tensor_copy(biasb[:], biasb_ps[:])

            # ---- phase B: g = dinv * (x @ W + cW)   fp16 [128, R*10]
            g_sb = big.tile([P, R * K], F16)
            xT3 = xT[:].rearrange("p (h n) -> p h n", n=NPAD)
            for r in (range(R) if "B" in PHASES else []):
                hw_ps = psHW.tile([P, K], F32, space="PSUM", tag="hw")
                nc.tensor.matmul(hw_ps[:], lhsT=xT3[:, 0, r * P:(r + 1) * P],
                                 rhs=w_sb[:, 0:K], start=True, stop=False)
                nc.tensor.matmul(hw_ps[:], lhsT=xT3[:, 1, r * P:(r + 1) * P],
                                 rhs=w_sb[:, K:2 * K], start=False, stop=False)
                nc.tensor.matmul(hw_ps[:], lhsT=ones_row_h[:], rhs=cw_sb[:],
                                 start=False, stop=True)
                nc.vector.tensor_scalar_mul(
                    g_sb[:, r * K:(r + 1) * K], hw_ps[:], dinvn_sb[:, r:r + 1])

            # ---- packed bounce + all-gather (fp16, 2MB total), per window
            g_bounce = dram.tile([NPAD, K], F16)
            if "AG" in PHASES:
              for w in range(NWIN):
                nc.sync.dma_start(
                    g_bounce[WB[w]:WB[w + 1], :].rearrange(
                        "(r p) j -> p r j", p=P),
                    g_sb[:, RB[w] * K:RB[w + 1] * K].rearrange(
                        "p (r j) -> p r j", j=K))
            # ---- per-window all-gather + expand into strided tables
            zrow = small.tile([1, 4 * K], F16)
            nc.vector.memset(zrow[:], 0.0)
            tables = []
            for w in range(NWIN):
                wrows = NCORES * QWS[w]
                tb = dram.tile([wrows + 1, P], F16, tag=f"tbl{w}")
                tables.append(tb)
                ga = dram.tile([wrows, K], F16, addr_space="Shared",
                               tag=f"gall{w}")
                if "AG" not in PHASES:
                    continue
                nc.gpsimd.collective_compute(
                    "AllGather", mybir.AluOpType.bypass,
                    replica_groups=[list(range(NCORES))],
                    ins=[g_bounce[WB[w]:WB[w + 1], :].opt()],
                    outs=[ga[:].opt()],
                )
                nc.sync.dma_start(tb[0:wrows, 0:K], ga[:])
                nc.sync.dma_start(tb[wrows:wrows + 1, 0:K],
                                  zrow[:, w * K:(w + 1) * K])

            # ---- per-window resident idx streams (wrapped int16)
            maxT16 = max(max(T16s), 1)
            winA = big.tile([P, maxT16], I16, tag="winA")
            winB = big.tile([P, maxT16], I16, tag="winB")
            for b0 in range(32, P, 32):
                nc.vector.memset(winA[b0:b0 + 32, :], 0)
                nc.vector.memset(winB[b0:b0 + 32, :], 0)

            # ---- windowed gathers + per-(window,row) partial reduces
            partials = []
            for w in range(NWIN):
                pt = big.tile([P, R * K], F32, tag=f"part{w}")
                nc.vector.memset(pt[:], 0.0)
                partials.append(pt)
            agg_sb = big.tile([P, R * K], F32)

            if "GATHER" in PHASES:
                loaded_w = set()
                for (w, rows, col0, cols) in chunks:
                    if w not in loaded_w:
                        loaded_w.add(w)
                        wt = winA if w % 2 == 0 else winB
                        t16 = T16s[w]
                        nc.sync.dma_start(wt[0:16, 0:t16],
                                          idx_in[:, wtok0[w] // 16:
                                                 wtok0[w] // 16 + t16])
                        nc.sync.dma_start(wt[16:32, 0:t16],
                                          idx_in[:, wtok0[w] // 16:
                                                 wtok0[w] // 16 + t16])
                    wt = winA if w % 2 == 0 else winB
                    n = cols * P
                    lc0 = col0 * P // 16 - wtok0[w] // 16
                    msg = msgp.tile([P, GCOLS * K], F16, tag="msgbuf")
                    _raw_dma_gather(
                        nc.gpsimd,
                        msg[:, 0:cols * K].rearrange("p (c j) -> p c j", j=K),
                        tables[w][:, 0:K], wt[:, lc0:lc0 + n // 16], n, K, P)
                    off = 0
                    i = 0
                    while i < len(rows):
                        kk = int(K_w[w, rows[i]])
                        j = i
                        while (j + 1 < len(rows)
                               and rows[j + 1] == rows[j] + 1
                               and int(K_w[w, rows[j + 1]]) == kk):
                            j += 1
                        nrun = j - i + 1
                        nc.vector.tensor_reduce(
                            out=partials[w][:, rows[i] * K:
                                            (rows[i] + nrun) * K].rearrange(
                                "p (n j) -> p n j", j=K),
                            in_=msg[:, off * K:(off + nrun * kk) * K].rearrange(
                                "p (n k j) -> p n j k", j=K, k=kk),
                            axis=mybir.AxisListType.X,
                            op=mybir.AluOpType.add)
                        off += nrun * kk
                        i = j + 1

            if "GATHER" in PHASES:
              nc.vector.tensor_add(out=partials[0][:], in0=partials[0][:],
                                 in1=partials[1][:])
              nc.vector.tensor_add(out=partials[2][:], in0=partials[2][:],
                                   in1=partials[3][:])
              nc.vector.tensor_add(out=agg_sb[:], in0=partials[0][:],
                                   in1=partials[2][:])

            # ---- logits = dinv*agg + bias ; softmax -> s (fp16)
            lg = big.tile([P, R * K], F32)
            lg3 = lg[:].rearrange("p (r j) -> p r j", j=K)
            if "SOFTMAX" in PHASES:
              nc.vector.tensor_tensor(
                out=lg3,
                in0=agg_sb[:].rearrange("p (r j) -> p r j", j=K),
                in1=dinv_sb[:].unsqueeze(2).to_broadcast([P, R, K]),
                op=mybir.AluOpType.mult)
              nc.vector.tensor_tensor(
                  out=lg3, in0=lg3,
                  in1=biasb[:].unsqueeze(1).to_broadcast([P, R, K]),
                  op=mybir.AluOpType.add)
              nc.scalar.activation(lg[:], lg[:],
                                   mybir.ActivationFunctionType.Exp)
            den = small.tile([P, R], F32)
            s_sb = big.tile([P, R * K], F16)
            if "SOFTMAX" in PHASES:
              nc.vector.tensor_reduce(out=den[:], in_=lg3,
                                      axis=mybir.AxisListType.X,
                                      op=mybir.AluOpType.add)
              nc.vector.reciprocal(den[:], den[:])
              nc.vector.tensor_tensor(
                  out=s_sb[:].rearrange("p (r j) -> p r j", j=K),
                  in0=lg3,
                  in1=den[:].unsqueeze(2).to_broadcast([P, R, K]),
                  op=mybir.AluOpType.mult)

            # ---- un-permute s from sigma order back to natural rank order
            s_nat = big.tile([P, R * K], F16, tag="snat")
            if "D" in PHASES:
                s_tbl = dram.tile([NPAD, P], F16, tag="stbl")
                nc.sync.dma_start(
                    s_tbl[:, 0:K].rearrange("(r p) j -> p r j", p=P),
                    s_sb[:].rearrange("p (r j) -> p r j", j=K))
                uidx_sb = small.tile([P, NPAD // 16], I16)
                nc.sync.dma_start(uidx_sb[0:32, :], uidx_in[:])
                for b0 in range(32, P, 32):
                    nc.vector.memset(uidx_sb[b0:b0 + 32, :], 0)
                _raw_dma_gather(
                    nc.gpsimd,
                    s_nat[:].rearrange("p (c j) -> p c j", j=K),
                    s_tbl[:, 0:K], uidx_sb[:, :], NPAD, K, P)

            # ---- p = s @ emb ; out = x + p  (fp16 out, 7 rows per DMA batch)
            DST = 7
            for r0 in (range(0, R, DST) if "D" in PHASES else []):
                nr = min(DST, R - r0)
                xload = opsp.tile([P, DST * IN_CH], F16, tag="xload")
                nc.sync.dma_start(
                    xload[:, 0:nr * IN_CH].rearrange(
                        "p (i ch) -> p i ch", ch=IN_CH),
                    x_in[r0 * P:(r0 + nr) * P, :].rearrange(
                        "(i p) ch -> p i ch", p=P))
                ostage = opsp2.tile([P, DST * IN_CH], F16, tag="ostage")
                for i in range(nr):
                    r = r0 + i
                    st_ps = psST.tile([K, P], F16, space="PSUM", tag="stps")
                    nc.tensor.transpose(st_ps[:], s_nat[:, r * K:(r + 1) * K],
                                        ident_h[:])
                    st = opsp.tile([K, P], F16, tag="st")
                    nc.vector.tensor_copy(st[:], st_ps[:])
                    p_ps = psP.tile([P, IN_CH], F32, space="PSUM", tag="pps")
                    nc.tensor.matmul(p_ps[:], lhsT=st[:], rhs=emb_sb[:],
                                     start=True, stop=True)
                    nc.vector.tensor_add(
                        out=ostage[:, i * IN_CH:(i + 1) * IN_CH],
                        in0=p_ps[:],
                        in1=xload[:, i * IN_CH:(i + 1) * IN_CH])
                nc.sync.dma_start(
                    out[r0 * P:(r0 + nr) * P, :].rearrange(
                        "(i p) ch -> p i ch", p=P),
                    ostage[:, 0:nr * IN_CH].rearrange(
                        "p (i ch) -> p i ch", ch=IN_CH))

    nc.compile()
    _BUILD_CACHE[key] = nc
    return nc


# ----------------------------------------------------------------------------
# Entry point
# ----------------------------------------------------------------------------

def kernel(x, edge_index, batch, W, bias, cluster_emb):
    x = np.asarray(x)
    W16 = np.asarray(W, dtype=np.float16)
    bias = np.asarray(bias, dtype=np.float32).reshape(1, K)
    emb16 = np.asarray(cluster_emb, dtype=np.float16)

    plan = host_prep_cached(edge_index)
    nc = build_kernel(plan["K_w"], plan["TOT16"])

    x16 = np.zeros((NCORES, NPAD, IN_CH), dtype=np.float16)
    x16[:, :NPC] = x.reshape(NCORES, NPC, IN_CH)

    in_maps = []
    for c in range(NCORES):
        in_maps.append({
            "x": x16[c],
            "w": W16,
            "bias": bias,
            "emb": emb16,
            "deg": plan["deg"][c],
            "degn": plan["deg_nat"][c],
            "idx": plan["idx"][c],
            "uidx": plan["uidx"][c],
        })

    res = run_bass_kernel_spmd(nc, in_maps, core_ids=list(range(NCORES)))

    out = np.empty((N_NODES, IN_CH), dtype=np.float32)
    for c in range(NCORES):
        out[c * NPC:(c + 1) * NPC] = res.results[c]["out"][:NPC]
    return out
